# revision 4
# baseline (speedup 1.0000x reference)
"""Trainium2 Bass kernel for a 3-cell LSTM decoder step (nn_AR_Decoder).

reference semantics (per sample):
    in_frame = kps @ W_emb.T + b_emb
    h0n,c0n = LSTMCell(in_frame, h0, c0; w_ih1,w_hh1,b_ih1,b_hh1)
    h1n,c1n = LSTMCell(h0,       h1, c1; w_ih2,w_hh2,b_ih2,b_hh2)
    h2n,c2n = LSTMCell(h1,       h2, c2; w_ih3,w_hh3,b_ih3,b_hh3)
    return (h2n, h0n, h1n, h2n, c0n, c1n, c2n)

Strategy: data-parallel over 8 NeuronCores (512 samples each). On device
everything lives in a transposed [feature, batch] layout so that
  * matmuls put the contraction dim (input features) on SBUF partitions,
    weights are the stationary operand, activations stream with N=512,
  * the per-gate bias is a per-partition scalar fused into the ACT pass,
  * no transposes are needed on device (host transposes in/out instead).
Weight columns are permuted on host so the 4 gates of one 128-row h-chunk
are contiguous; each group of 4 PSUM banks then yields i/f/g/o for one
h-chunk, which the DVE combines into c_new/h_new.
Matmuls run in bf16 (fp32 PSUM accumulation).
"""

import sys

import numpy as np

for _p in ("/opt/trn_rl_repo", "/root/.axon_site/_ro/trn_rl_repo"):
    if _p not in sys.path:
        sys.path.append(_p)

import ml_dtypes

N_CORES = 8
B_FULL = 4096
B = B_FULL // N_CORES  # 512 per core
KPS = 256
HID = 1024
G4 = 4 * HID
P = 128
KC_KPS = KPS // P  # 2
KC_HID = HID // P  # 8
NJ = KC_HID  # 8 h-chunks per cell
BF16 = ml_dtypes.bfloat16

_CACHE = {}


def _build_bass():
    import concourse.tile as tile
    from concourse import bacc, mybir

    f32 = mybir.dt.float32
    bf16 = mybir.dt.bfloat16
    AF = mybir.ActivationFunctionType

    nc = bacc.Bacc("TRN2", target_bir_lowering=False, debug=False,
                   num_devices=N_CORES)

    # ---- DRAM parameters (per-core shapes) ----
    kpsT = nc.dram_tensor("kpsT", [KPS, B], bf16, kind="ExternalInput").ap()
    hT = [nc.dram_tensor(f"h{l}T", [HID, B], bf16, kind="ExternalInput").ap()
          for l in range(3)]
    cT = [nc.dram_tensor(f"c{l}T", [HID, B], f32, kind="ExternalInput").ap()
          for l in range(3)]
    wembT = nc.dram_tensor("wembT", [KPS, KPS], bf16, kind="ExternalInput").ap()
    bemb = nc.dram_tensor("bemb", [P, KC_KPS], f32, kind="ExternalInput").ap()
    kcx_l = [KC_KPS, KC_HID, KC_HID]
    wx = [nc.dram_tensor(f"wx{l}", [kcx_l[l] * P, G4], bf16,
                         kind="ExternalInput").ap() for l in range(3)]
    wh = [nc.dram_tensor(f"wh{l}", [HID, G4], bf16,
                         kind="ExternalInput").ap() for l in range(3)]
    bias = [nc.dram_tensor(f"b{l}", [P, 4 * NJ], f32,
                           kind="ExternalInput").ap() for l in range(3)]
    hoT = [nc.dram_tensor(f"h{l}nT", [HID, B], f32, kind="ExternalOutput").ap()
           for l in range(3)]
    coT = [nc.dram_tensor(f"c{l}nT", [HID, B], f32, kind="ExternalOutput").ap()
           for l in range(3)]

    with tile.TileContext(nc) as tc:
        with (
            tc.tile_pool(name="acts", bufs=1) as acts_pool,
            tc.tile_pool(name="wpool", bufs=3) as wpool,
            tc.tile_pool(name="cpool", bufs=3) as cpool,
            tc.tile_pool(name="gates", bufs=2) as gates_pool,
            tc.tile_pool(name="ew", bufs=2) as ew_pool,
            tc.tile_pool(name="psum", bufs=8, space="PSUM") as psum_pool,
        ):
            # ---- persistent SBUF loads ----
            kps_sb = acts_pool.tile([P, KC_KPS, B], bf16)
            nc.sync.dma_start(kps_sb[:], kpsT.rearrange("(kc p) b -> p kc b", p=P))
            h_sb = []
            for l in range(3):
                t = acts_pool.tile([P, KC_HID, B], bf16, name=f"h{l}sb")
                nc.sync.dma_start(t[:], hT[l].rearrange("(kc p) b -> p kc b", p=P))
                h_sb.append(t)
            bemb_sb = acts_pool.tile([P, KC_KPS], f32)
            nc.sync.dma_start(bemb_sb[:], bemb)
            bias_sb = []
            for l in range(3):
                t = acts_pool.tile([P, 4 * NJ], f32, name=f"b{l}sb")
                nc.sync.dma_start(t[:], bias[l])
                bias_sb.append(t)
            wemb_sb = acts_pool.tile([P, KC_KPS, KPS], bf16)
            nc.sync.dma_start(wemb_sb[:], wembT.rearrange("(kc p) m -> p kc m", p=P))

            # ---- embedding: in_frameT = W_emb @ kpsT + b_emb ----
            inframe_sb = acts_pool.tile([P, KC_KPS, B], bf16)
            for m in range(KC_KPS):
                ps = psum_pool.tile([P, B], f32, tag="ps", name=f"ps_emb{m}")
                for k in range(KC_KPS):
                    nc.tensor.matmul(ps[:], wemb_sb[:, k, m * P:(m + 1) * P],
                                     kps_sb[:, k, :],
                                     start=(k == 0), stop=(k == KC_KPS - 1))
                nc.scalar.activation(inframe_sb[:, m, :], ps[:], AF.Identity,
                                     bias=bemb_sb[:, m:m + 1])

            # ---- the 3 LSTM cells ----
            # (x source tile, #x k-chunks, h source tile) per cell
            specs = [
                (inframe_sb, KC_KPS, h_sb[0]),
                (h_sb[0], KC_HID, h_sb[1]),
                (h_sb[1], KC_HID, h_sb[2]),
            ]
            for l, (xsb, kcx, hsb) in enumerate(specs):
                n_k = kcx + KC_HID
                wxr = wx[l].rearrange("(kc p) m -> p kc m", p=P)
                whr = wh[l].rearrange("(kc p) m -> p kc m", p=P)
                cr = cT[l].rearrange("(kc p) b -> p kc b", p=P)
                hor = hoT[l].rearrange("(kc p) b -> p kc b", p=P)
                cor = coT[l].rearrange("(kc p) b -> p kc b", p=P)
                for j in range(NJ):
                    cols = slice(j * 4 * P, (j + 1) * 4 * P)
                    wg = wpool.tile([P, n_k, 4 * P], bf16, tag="wg",
                                    name=f"wg_{l}_{j}")
                    nc.sync.dma_start(wg[:, :kcx, :], wxr[:, :, cols])
                    nc.sync.dma_start(wg[:, kcx:, :], whr[:, :, cols])
                    c_sb = cpool.tile([P, B], f32, tag="c", name=f"c_{l}_{j}")
                    nc.sync.dma_start(c_sb[:], cr[:, j, :])

                    pss = [psum_pool.tile([P, B], f32, tag="ps",
                                          name=f"ps{q}_{l}_{j}")
                           for q in range(4)]
                    for k in range(n_k):
                        rhs = xsb[:, k, :] if k < kcx else hsb[:, k - kcx, :]
                        for q in range(4):
                            nc.tensor.matmul(pss[q][:],
                                             wg[:, k, q * P:(q + 1) * P], rhs,
                                             start=(k == 0),
                                             stop=(k == n_k - 1))
                    # gates: q=0 i(sig), 1 f(sig), 2 g(tanh), 3 o(sig)
                    gt = []
                    for q in range(4):
                        func = AF.Tanh if q == 2 else AF.Sigmoid
                        t = gates_pool.tile([P, B], f32, tag=f"g{q}",
                                            name=f"gate{q}_{l}_{j}")
                        bcol = j * 4 + q
                        nc.scalar.activation(t[:], pss[q][:], func,
                                             bias=bias_sb[l][:, bcol:bcol + 1])
                        gt.append(t)
                    fc = ew_pool.tile([P, B], f32, tag="fc", name=f"fc_{l}_{j}")
                    nc.vector.tensor_mul(fc[:], gt[1][:], c_sb[:])
                    ig = ew_pool.tile([P, B], f32, tag="ig", name=f"ig_{l}_{j}")
                    nc.vector.tensor_mul(ig[:], gt[0][:], gt[2][:])
                    cn = ew_pool.tile([P, B], f32, tag="cn", name=f"cn_{l}_{j}")
                    nc.vector.tensor_add(cn[:], fc[:], ig[:])
                    nc.sync.dma_start(cor[:, j, :], cn[:])
                    th = ew_pool.tile([P, B], f32, tag="th", name=f"th_{l}_{j}")
                    nc.scalar.activation(th[:], cn[:], AF.Tanh)
                    hn = ew_pool.tile([P, B], f32, tag="hn", name=f"hn_{l}_{j}")
                    nc.vector.tensor_mul(hn[:], gt[3][:], th[:])
                    nc.sync.dma_start(hor[:, j, :], hn[:])
    nc.compile()
    return nc


def _get_nc():
    if "nc" not in _CACHE:
        _CACHE["nc"] = _build_bass()
    return _CACHE["nc"]


# column permutation: new col (j*4+q)*128+t  <-  orig col q*1024+j*128+t
_PERM = np.arange(G4).reshape(4, NJ, P).transpose(1, 0, 2).reshape(-1)


def _prep_shared(inputs):
    """Host-side packing of the replicated weights."""
    f32 = np.float32

    def wT_perm(w):  # [G4, K] -> [K, G4] bf16, gate-interleaved columns
        return np.ascontiguousarray(w.T[:, _PERM]).astype(BF16)

    def b_pack(b):  # [G4] -> [128, 32] f32, col m' = j*4+q
        return np.ascontiguousarray(b[_PERM].reshape(4 * NJ, P).T).astype(f32)

    shared = {
        "wembT": np.ascontiguousarray(inputs["W_emb"].T).astype(BF16),
        "bemb": np.ascontiguousarray(
            inputs["b_emb"].reshape(KC_KPS, P).T).astype(f32),
    }
    for l, sfx in enumerate("123"):
        shared[f"wx{l}"] = wT_perm(inputs[f"w_ih{sfx}"])
        shared[f"wh{l}"] = wT_perm(inputs[f"w_hh{sfx}"])
        shared[f"b{l}"] = b_pack(inputs[f"b_ih{sfx}"] + inputs[f"b_hh{sfx}"])
    return shared


def _make_in_maps(inputs):
    shared = _prep_shared(inputs)
    in_maps = []
    for c in range(N_CORES):
        sl = slice(c * B, (c + 1) * B)
        m = dict(shared)
        m["kpsT"] = np.ascontiguousarray(inputs["kps"][sl].T).astype(BF16)
        for l in range(3):
            m[f"h{l}T"] = np.ascontiguousarray(
                inputs[f"h{l}"][sl].T).astype(BF16)
            m[f"c{l}T"] = np.ascontiguousarray(
                inputs[f"c{l}"][sl].T).astype(np.float32)
        in_maps.append(m)
    return in_maps


def _assemble(res):
    hn = [np.empty((B_FULL, HID), np.float32) for _ in range(3)]
    cn = [np.empty((B_FULL, HID), np.float32) for _ in range(3)]
    for c in range(N_CORES):
        sl = slice(c * B, (c + 1) * B)
        for l in range(3):
            hn[l][sl] = res[c][f"h{l}nT"].T
            cn[l][sl] = res[c][f"c{l}nT"].T
    return (hn[2], hn[0], hn[1], hn[2], cn[0], cn[1], cn[2])


def kernel(**inputs):
    from concourse.bass_utils import run_bass_kernel_spmd

    inputs = {k: np.asarray(v) for k, v in inputs.items()}
    nc = _get_nc()
    in_maps = _make_in_maps(inputs)
    res = run_bass_kernel_spmd(nc, in_maps, list(range(N_CORES))).results
    return _assemble(res)
